# revision 1
# baseline (speedup 1.0000x reference)
"""Trainium2 Bass kernel for nn_ASCNet (sparse axial attention net).

Data-parallel over batch: 8 NeuronCores, 1 image each. Full inputs in,
full output out. Heavy matmuls in bf16 with fp32 PSUM accumulation; the
CCAM `e` path is kept fp32-accurate via hi/lo bf16 splits because its
softmax is near-argmin and tie-sensitive.

Layout: activations as [channel partitions, spatial free], spatial
row-major (n = h*128 + w).
"""

import os
import sys

sys.path.insert(0, "/opt/trn_rl_repo")

import numpy as np
import ml_dtypes

import concourse.bass as bass
import concourse.mybir as mybir
import concourse.tile as tile

BF16 = mybir.dt.bfloat16
F32 = mybir.dt.float32
AF = mybir.ActivationFunctionType
ALU = mybir.AluOpType
AX = mybir.AxisListType

H = W = 128
N = H * W
WP = W + 2
NPAD = (H + 2) * WP
CHUNK = 512
NCHUNK = N // CHUNK
SCALE = 16 ** -0.5

DEBUG = bool(int(os.environ.get("KDEBUG", "0")))
DW_PE_TAPS = int(os.environ.get("DW_PE_TAPS", "9"))
DW_POOL_TAPS = int(os.environ.get("DW_POOL_TAPS", "0"))


# ---- walrus workaround: split multi-wait instructions into sem-wait NOPs ----
import orjson as _orjson


def _split_waits_json(bir_bytes):
    d = _orjson.loads(bir_bytes)
    uid = [0]
    for f in d.get("functions", []):
        for bb in f.get("blocks", []):
            insts = bb.get("instructions", [])
            out = []
            for ins in insts:
                si = ins.get("sync_info") or {}
                waits = si.get("on_wait") or []
                if len(waits) > 1:
                    eng = ins.get("engine")
                    dbg = ins.get("debug")
                    for wv in waits[:-1]:
                        uid[0] += 1
                        nop = {
                            "engine": eng,
                            "ins": [],
                            "outs": [],
                            "name": f"wsplit_{uid[0]}",
                            "opcode": "EventSemaphore",
                            "sync_info": {"on_update": [], "on_wait": [wv]},
                        }
                        if dbg is not None:
                            nop["debug"] = dbg
                        out.append(nop)
                    si["on_wait"] = [waits[-1]]
                    ins["sync_info"] = si
                out.append(ins)
            bb["instructions"] = out
    return _orjson.dumps(d)


_orig_mtjb = mybir.module_to_json_bytes


def _patched_mtjb(m):
    return _split_waits_json(_orig_mtjb(m))


mybir.module_to_json_bytes = _patched_mtjb

TAPS = [(0, 0), (-1, -1), (-1, 0), (-1, 1), (0, -1), (0, 1),
        (1, -1), (1, 0), (1, 1)]


def _bf(a):
    return np.ascontiguousarray(np.asarray(a)).astype(ml_dtypes.bfloat16)


def _f32(a):
    return np.ascontiguousarray(np.asarray(a)).astype(np.float32)


def _interp_mat(n_in, n_out):
    M = np.zeros((n_out, n_in), np.float64)
    scale = n_in / n_out
    for i in range(n_out):
        c = (i + 0.5) * scale - 0.5
        lo = int(np.floor(c))
        frac = c - lo
        M[i, min(max(lo, 0), n_in - 1)] += 1 - frac
        M[i, min(max(lo + 1, 0), n_in - 1)] += frac
    return M.astype(np.float32)


def _ap(t, off, dims, parts=None):
    p0 = list(t.ap[0])
    if parts is not None:
        p0 = [p0[0], parts]
    return bass.AP(tensor=t.tensor, offset=t.offset + off,
                   ap=[p0] + [[s, c] for (s, c) in dims])


def build_nc():
    nc = bass.Bass()
    T = {}

    def din(name, shape, dtype):
        t = nc.dram_tensor(name, list(shape), dtype, kind="ExternalInput")
        T[name] = t[:]

    din("xstack", (128, NPAD), BF16)
    din("wccp", (128, 3, 128), BF16)
    din("wccs", (64, 3, 128), BF16)
    din("bcc", (128, 1), F32)
    din("wenT_h", (128, 8), BF16)
    din("wenT_l", (128, 8), BF16)
    din("ben", (8, 1), F32)
    din("ident_b", (128, 128), BF16)
    din("ident_f", (128, 128), F32)
    din("wqT", (128, 128), BF16)
    din("wkT", (128, 128), BF16)
    din("wvT", (128, 256), BF16)
    din("bq", (128, 1), F32)
    din("bk", (128, 1), F32)
    din("bv", (128, 1), F32)
    din("bv2", (128, 1), F32)
    din("wqsT", (128, 128), BF16)
    din("wksT", (128, 128), BF16)
    din("wvsT", (128, 256), BF16)
    din("bq2", (16, 8), F32)
    din("bk2", (16, 8), F32)
    din("posqk_r", (16, 8192), BF16)   # cols (h, q/k, j, L)
    din("posqk_c", (16, 8192), BF16)
    din("bvT", (128, 256), BF16)
    din("ones1", (128, 1), BF16)
    din("ones2", (1, 128), F32)
    din("wdw", (128, 4, 9), F32)
    din("bd", (128, 4), F32)
    din("wdiag", (128, 4, 9, 128), BF16)
    din("wpT", (128, 4, 128), BF16)
    din("bp6", (128, 1), F32)
    din("wrT", (128, 2, 256), BF16)
    din("br", (128, 2), F32)
    din("wc2T", (128, 2, 256), BF16)
    din("bc2", (128, 2), F32)
    din("woT", (128, 2, 128), BF16)
    din("bo3", (128, 1), F32)

    T["out"] = nc.dram_tensor("out", [128, N], F32, kind="ExternalOutput")[:]
    if DEBUG:
        for nm, shp in (("d_cb", (128, N)), ("d_xs", (128, N)),
                        ("d_z0", (128, N)), ("d_qkv", (128, N)),
                        ("d_e", (128, 8)), ("d_sxr", (128, 512)),
                        ("d_xr3", (128, 1024)), ("d_qf", (128, N))):
            T[nm] = nc.dram_tensor(nm, list(shp), F32, kind="ExternalOutput")[:]

    with tile.TileContext(nc) as tc:
        _emit(nc, tc, T)
    return nc


def _emit(nc, tc, T):
    ctxs = []

    def pool(name, bufs, space="SBUF"):
        p = tc.tile_pool(name=name, bufs=bufs, space=space)
        ctxs.append(p)
        return p.__enter__()

    const = pool("const", 1)
    big = pool("big", 1)
    small = pool("small", 1)
    work = pool("work", 2)
    psA = pool("psA", 3, "PSUM")
    psB = pool("psB", 2, "PSUM")
    psC = pool("psC", 2, "PSUM")
    psE = pool("psE", 1, "PSUM")

    def mmtile(shape=(128, CHUNK), dt=F32):
        return psA.tile(list(shape), dt, tag="mm", name="mm")

    C = {}
    for name in ("wccp", "wccs", "bcc", "wenT_h", "wenT_l", "ben", "ident_b",
                 "ident_f", "wqT", "wkT", "wvT", "bq", "bk", "bv", "bv2",
                 "wqsT", "wksT", "wvsT", "bq2", "bk2", "bvT", "ones1",
                 "ones2", "wdw", "bd", "wdiag", "wpT", "bp6", "wrT", "br",
                 "wc2T", "bc2", "woT", "bo3"):
        src = T[name]
        t = const.tile(list(src.shape), src.dtype, tag=name, name=name)
        nc.gpsimd.dma_start(out=t, in_=src)
        C[name] = t

    # big slots (tag -> lifetimes):
    #  bigA: xstack(s1) -> qkv band tiles (s6)
    #  bigB: cb(s1-3)   -> vf0(s6-8)
    #  bigC: cbT(s3)    -> vf1(s6-8)
    #  bigD: xs(s3-6)
    xstack = big.tile([128, NPAD], BF16, tag="bigA", name="xstack")
    nc.sync.dma_start(out=xstack, in_=T["xstack"])

    def dump(name, src_tile, ncols):
        if not DEBUG:
            return
        for c0 in range(0, ncols, 512):
            w = min(512, ncols - c0)
            tmp = work.tile([128, w], F32, tag="dbg", name="dbg")
            nc.vector.tensor_copy(tmp, src_tile[:, c0:c0 + w])
            nc.sync.dma_start(out=T[name][:, c0:c0 + w], in_=tmp)

    # ======== stage 1: conv3x3 64->128 -> cb (bf16); cbT via xbar DMA ====
    cb = big.tile([128, N], BF16, tag="bigB", name="cb")
    cbT = big.tile([128, N], BF16, tag="bigC", name="cbT")
    cbT3 = cbT.rearrange("p (t j) -> p t j", j=128)
    for c in range(NCHUNK):
        h0 = c * 4
        pt = mmtile()
        for di in range(3):
            rhs = _ap(xstack, (h0 + di) * WP, [(WP, 4), (1, 128)])
            nc.tensor.matmul(pt, C["wccp"][:, di, :], rhs,
                             start=(di == 0), stop=False)
        for di in range(3):
            rhs = _ap(xstack, (h0 + di) * WP + 2, [(WP, 4), (1, 128)], parts=64)
            nc.tensor.matmul(pt, C["wccs"][:, di, :], rhs,
                             start=False, stop=(di == 2))
        nc.scalar.activation(cb[:, c * CHUNK:(c + 1) * CHUNK], pt, AF.Relu,
                             bias=C["bcc"])
        if c % 8 == 7:
            k = c // 8
            eng = nc.sync if k % 2 == 0 else nc.scalar
            eng.dma_start_transpose(
                cbT3[:, k * 32:(k + 1) * 32, :],
                cb[:, k * 4096:(k + 1) * 4096])
    dump("d_cb", cb, N)

    cfT_h = small.tile([128, 8 * 128], BF16, tag="cfT_h", name="cfT_h")
    cfT_l = small.tile([128, 8 * 128], BF16, tag="cfT_l", name="cfT_l")
    for g in range(2):
        ptT = psB.tile([128, 512], F32, tag="tp", name="tpf")
        for cc_ in range(16):
            c = g * 16 + cc_
            pc = psC.tile([8, CHUNK], F32, tag="oc", name="oc")
            nc.tensor.matmul(pc, C["wenT_h"], cb[:, c * CHUNK:(c + 1) * CHUNK],
                             start=True, stop=False)
            nc.tensor.matmul(pc, C["wenT_l"], cb[:, c * CHUNK:(c + 1) * CHUNK],
                             start=False, stop=True)
            cf32c = work.tile([8, CHUNK], F32, tag="cf32c", name="cf32c", bufs=1)
            nc.scalar.activation(cf32c, pc, AF.Relu, bias=C["ben"])
            for i in range(4):
                nc.tensor.transpose(ptT[:, (cc_ * 4 + i) * 8:(cc_ * 4 + i + 1) * 8],
                                    cf32c[:, i * 128:(i + 1) * 128],
                                    C["ident_f"][0:8, 0:8])
        sl = slice(g * 512, (g + 1) * 512)
        nc.scalar.activation(cfT_h[:, sl], ptT, AF.Copy)
        nc.vector.scalar_tensor_tensor(cfT_l[:, sl], ptT, 1.0, cfT_h[:, sl],
                                       ALU.mult, ALU.subtract)

    # ======== stage 3: e, softmax, dec, xs ========
    pe_ = psE.tile([128, 8], F32, tag="e", name="pe")
    for t in range(N // 128):
        sl = slice(t * 128, (t + 1) * 128)
        kl = slice(t * 8, (t + 1) * 8)
        nc.tensor.matmul(pe_, cbT[:, sl], cfT_h[:, kl],
                         start=(t == 0), stop=False)
        nc.tensor.matmul(pe_, cbT[:, sl], cfT_l[:, kl], start=False,
                         stop=(t == N // 128 - 1))
    if DEBUG:
        tmpe = work.tile([128, 8], F32, tag="dbge", name="dbge")
        nc.vector.tensor_copy(tmpe, pe_)
        nc.sync.dma_start(out=T["d_e"], in_=tmpe)
    emin = small.tile([128, 1], F32, tag="emin", name="emin")
    nc.vector.tensor_reduce(emin, pe_, AX.X, ALU.min)
    a_sb = small.tile([128, 8], F32, tag="a_sb", name="a_sb")
    nc.scalar.activation(a_sb, pe_, AF.Exp, bias=emin, scale=-1.0)
    asum = small.tile([128, 1], F32, tag="asum", name="asum")
    nc.vector.tensor_reduce(asum, a_sb, AX.X, ALU.add)
    arec = small.tile([128, 1], F32, tag="arec", name="arec")
    nc.vector.reciprocal(arec, asum)
    a16 = small.tile([128, 8], BF16, tag="a16", name="a16")
    nc.vector.tensor_scalar(a16, a_sb, arec, None, ALU.mult)
    paT = psC.tile([8, 128], BF16, tag="oc", name="paT")
    nc.tensor.transpose(paT, a16, C["ident_b"])
    aT = small.tile([8, 128], BF16, tag="aT_sb", name="aT_sb")
    nc.scalar.activation(aT, paT, AF.Copy)

    xs = big.tile([128, N], BF16, tag="bigD", name="xs")
    sxr32 = work.tile([128, 4, 128], F32, tag="sx32r", name="sxr32", bufs=1)
    psxc = psE.tile([128, 512], F32, tag="e", name="psxc")
    for c in range(NCHUNK):
        sl = slice(c * CHUNK, (c + 1) * CHUNK)
        pc = psC.tile([8, CHUNK], F32, tag="oc", name="occ")
        nc.tensor.matmul(pc, C["wenT_h"], cb[:, sl], start=True, stop=False)
        nc.tensor.matmul(pc, C["wenT_l"], cb[:, sl], start=False, stop=True)
        cf16c = work.tile([8, CHUNK], BF16, tag="cf16c", name="cf16c")
        nc.scalar.activation(cf16c, pc, AF.Relu, bias=C["ben"])
        pt = mmtile()
        nc.tensor.matmul(pt, aT, cf16c, start=True, stop=True)
        nc.vector.scalar_tensor_tensor(xs[:, sl], pt, 0.5, cb[:, sl],
                                       ALU.mult, ALU.add)
        if c % 8 == 7:
            qv_ = c // 8
            # sxr partial: h-range [32q, 32q+32)
            src_ap = bass.AP(tensor=xs.tensor, offset=xs.offset + qv_ * 4096,
                             ap=[list(xs.ap[0]), [1, 4], [128, 32], [4, 32]])
            dst_ap = bass.AP(tensor=sxr32.tensor,
                             offset=sxr32.offset + qv_ * 32,
                             ap=[list(sxr32.ap[0]), [128, 4], [1, 32]])
            nc.vector.tensor_reduce(dst_ap, src_ap, AX.X, ALU.add)
            # sxc partial: g-range [8q, 8q+8)
            pass
        for jr in range(4):
            h = 4 * c + jr
            nc.tensor.matmul(psxc[:, (h % 4) * 128:(h % 4) * 128 + 128],
                             C["ident_b"], xs[:, h * 128:(h + 1) * 128],
                             start=(h < 4), stop=(h >= 124),
                             skip_group_check=True)
        dump("d_xs", xs, N)

    # ======== stage 4: finish shunts + shunt-v (svT) ========
    sxr = small.tile([128, 512], BF16, tag="sxr", name="sxr")
    nc.vector.tensor_copy(sxr, sxr32.rearrange("p a b -> p (a b)"))
    sxc = small.tile([128, 512], BF16, tag="sxc", name="sxc")
    nc.vector.tensor_copy(sxc, psxc)
    dump("d_sxr", sxr, 512)

    def shunt_v(sx, tagp):
        svT = small.tile([128, 4, 256], BF16, tag=f"svT{tagp}", name="svT")
        for j in range(4):
            pv = mmtile((128, 256))
            nc.tensor.matmul(pv, sx[:, j * 128:(j + 1) * 128], C["wvsT"],
                             start=True, stop=True)
            nc.vector.scalar_tensor_tensor(svT[:, j, :], pv, 1.0, C["bvT"],
                                           ALU.mult, ALU.add)
        return svT

    svT_r = shunt_v(sxr, "r")
    svT_c = shunt_v(sxc, "c")

    # ======== stages 5/6/7 interleaved ========
    def attention_unit(sx, svT, posdram, tagp, xr_relu, h):
        post = work.tile([16, 1024], BF16, tag="post", name="post", bufs=1)
        nc.sync.dma_start(out=post, in_=posdram[:, h * 1024:(h + 1) * 1024])
        qk_h = work.tile([16, 1024], BF16, tag="qkh", name="qkh")
        pq2 = psC.tile([16, 512], F32, tag="oc", name="pq2")
        nc.tensor.matmul(pq2, C["wqsT"][:, h * 16:(h + 1) * 16], sx,
                         start=True, stop=True)
        nc.vector.scalar_tensor_tensor(qk_h[:, 0:512], pq2,
                                       C["bq2"][:, h:h + 1],
                                       post[:, 0:512], ALU.add, ALU.add)
        pk2 = psC.tile([16, 512], F32, tag="oc", name="pk2")
        nc.tensor.matmul(pk2, C["wksT"][:, h * 16:(h + 1) * 16], sx,
                         start=True, stop=True)
        nc.vector.scalar_tensor_tensor(qk_h[:, 512:1024], pk2,
                                       C["bk2"][:, h:h + 1],
                                       post[:, 512:1024], ALU.add, ALU.add)
        ht, hr = divmod(h, 4)
        for j in range(4):
            pl = psB.tile([128, 128], F32, tag="tp", name="pl")
            nc.tensor.matmul(pl, qk_h[:, 512 + j * 128: 512 + (j + 1) * 128],
                             qk_h[:, j * 128:(j + 1) * 128],
                             start=True, stop=True)
            expT = work.tile([128, 128], BF16, tag="expT", name="expT", bufs=4)
            nc.scalar.activation(expT, pl, AF.Exp)
            pz = psC.tile([1, 128], F32, tag="oc", name="pz")
            nc.tensor.matmul(pz, C["ones1"], expT, start=True, stop=True)
            zrec = work.tile([1, 128], F32, tag="zrec", name="zrec", bufs=4)
            nc.vector.reciprocal(zrec, pz)
            pzb = psB.tile([128, 128], F32, tag="tp", name="pzb")
            nc.tensor.matmul(pzb, C["ones2"], zrec, start=True, stop=True)
            expN = work.tile([128, 128], BF16, tag="expN", name="expN", bufs=4)
            nc.vector.tensor_mul(expN, expT, pzb)
            po = psC.tile([128, 128], F32, tag="oc", name="po")
            nc.tensor.matmul(po[hr * 32:(hr + 1) * 32, :],
                             svT[:, j, h * 32:(h + 1) * 32], expN,
                             start=True, stop=True,
                             tile_position=(0, hr * 32))
            nc.scalar.activation(
                xr_relu[ht][hr * 32:(hr + 1) * 32, j * 128:(j + 1) * 128],
                po[hr * 32:(hr + 1) * 32, :], AF.Relu)

    xrr = [small.tile([128, 512], BF16, tag=f"xrer{i}", name="xre")
           for i in range(2)]
    xcr = [small.tile([128, 512], BF16, tag=f"xrec{i}", name="xre")
           for i in range(2)]
    att_units = [(sxr, svT_r, T["posqk_r"], "r", xrr),
                 (sxc, svT_c, T["posqk_c"], "c", xcr)]
    att_queue = [(d, h) for h in range(8) for d in range(2)]

    def post_conv(xin, wT, bias_col, tagp):
        out_t = [small.tile([128, 512], BF16, tag=f"x3{tagp}{m}", name="x3")
                 for m in range(2)]
        for m in range(2):
            pt = mmtile()
            for kt in range(2):
                nc.tensor.matmul(pt, wT[:, kt, m * 128:(m + 1) * 128],
                                 xin[kt], start=(kt == 0), stop=(kt == 1))
            nc.scalar.activation(out_t[m], pt, AF.Identity,
                                 bias=bias_col[:, m:m + 1])
        return out_t

    # stage-6 setup
    dramp = pool("dram", 1, "DRAM")
    qkv32_dram = dramp.tile([128, N], F32, tag="qkv32", name="qkv32")
    vf0 = big.tile([128, N], BF16, tag="bigB", name="vf0")
    vf1 = big.tile([128, N], BF16, tag="bigC", name="vf1")
    vts = (vf0, vf1)

    if DW_POOL_TAPS == 2:
        POOL_SET = {2, 7}
    elif DW_POOL_TAPS == 1:
        POOL_SET = {7}
    else:
        POOL_SET = set()
    PE_SET = set(range(9)) - POOL_SET
    PE_SET = set(sorted(PE_SET)[:DW_PE_TAPS])

    def tap_ranges(c, di, dj):
        h0 = c * 4
        r_lo, r_hi = 0, 4
        if h0 + di < 0:
            r_lo = -di - h0
        if h0 + 3 + di > 127:
            r_hi = 4 - (h0 + 3 + di - 127)
        c_lo = max(0, -dj)
        c_hi = min(128, 128 - dj)
        return r_lo, r_hi - r_lo, c_lo, c_hi - c_lo

    wlhsT = (C["wqT"], C["wkT"], C["wvT"][:, 0:128], C["wvT"][:, 128:256])
    wbias = (C["bq"], C["bk"], C["bv"], C["bv2"])

    S7 = {}

    def emit_band(b):
        band_lo = max(0, 16 * b - 1)
        band_hi = min(128, 16 * b + 17)
        cols = (band_hi - band_lo) * 128
        band = big.tile([128, 4, 2304], BF16, tag="bigA", name="band")
        for ct in range(4):
            c0 = 0
            while c0 < cols:
                w = min(CHUNK, cols - c0)
                pt = mmtile()
                nc.tensor.matmul(pt[:, 0:w], wlhsT[ct],
                                 xs[:, band_lo * 128 + c0: band_lo * 128 + c0 + w],
                                 start=True, stop=True)
                nc.scalar.activation(band[:, ct, c0:c0 + w], pt[:, 0:w],
                                     AF.Identity, bias=wbias[ct])
                c0 += w
        core_off = (16 * b - band_lo) * 128
        for vt in range(2):
            nc.gpsimd.tensor_copy(
                vts[vt][:, 16 * b * 128: 16 * b * 128 + 2048],
                band[:, 2 + vt, core_off: core_off + 2048])
        if DEBUG and b == 0:
            tmpq = work.tile([128, 512], F32, tag="dbg", name="dbg")
            nc.vector.tensor_copy(tmpq, band[:, 0, core_off:core_off + 512])
            nc.sync.dma_start(out=T["d_qf"][:, 0:512], in_=tmpq)
        for q in range(4):
            c = 4 * b + q
            zts = []
            for ct in range(4):
                pe_taps = sorted(PE_SET)
                pool_taps = sorted(POOL_SET)
                dve_taps = [i for i in range(9)
                            if i not in PE_SET and i not in POOL_SET]
                pt = mmtile() if pe_taps else None
                for idx, ti in enumerate(pe_taps):
                    di, dj = TAPS[ti]
                    rl, rc, cl, cc = tap_ranges(c, di, dj)
                    off = (4 * q + (16 * b - band_lo) + rl + di) * 128 + cl + dj
                    rhs = _ap(band, ct * 2304 + off, [(128, rc), (1, cc)])
                    outp = _ap(pt, rl * 128 + cl, [(128, rc), (1, cc)])
                    nc.tensor.matmul(outp, C["wdiag"][:, ct, ti, :], rhs,
                                     start=(idx == 0),
                                     stop=(idx == len(pe_taps) - 1),
                                     skip_group_check=True)
                acc = None
                for ti in pool_taps:
                    di, dj = TAPS[ti]
                    rl, rc, cl, cc = tap_ranges(c, di, dj)
                    off = (4 * q + (16 * b - band_lo) + rl + di) * 128 + cl + dj
                    rhs = _ap(band, ct * 2304 + off, [(128, rc), (1, cc)])
                    if acc is None:
                        acc = work.tile([128, CHUNK], BF16, tag="accp",
                                        name="accp", bufs=2)
                        if (rl, rc, cl, cc) != (0, 4, 0, 128):
                            nc.gpsimd.memset(acc, 0.0)
                        nc.gpsimd.tensor_scalar(
                            _ap(acc, rl * 128 + cl, [(128, rc), (1, cc)]),
                            rhs, C["wdw"][:, ct, ti:ti + 1], None, ALU.mult)
                    else:
                        accv = _ap(acc, rl * 128 + cl, [(128, rc), (1, cc)])
                        nc.gpsimd.scalar_tensor_tensor(
                            accv, rhs, C["wdw"][:, ct, ti:ti + 1], accv,
                            ALU.mult, ALU.add)
                for ti in dve_taps:
                    di, dj = TAPS[ti]
                    rl, rc, cl, cc = tap_ranges(c, di, dj)
                    off = (4 * q + (16 * b - band_lo) + rl + di) * 128 + cl + dj
                    rhs = _ap(band, ct * 2304 + off, [(128, rc), (1, cc)])
                    if acc is None:
                        acc = work.tile([128, CHUNK], BF16, tag="accp",
                                        name="accp", bufs=2)
                        if (rl, rc, cl, cc) != (0, 4, 0, 128):
                            nc.vector.memset(acc, 0.0)
                        nc.vector.tensor_scalar(
                            _ap(acc, rl * 128 + cl, [(128, rc), (1, cc)]),
                            rhs, C["wdw"][:, ct, ti:ti + 1], None, ALU.mult)
                    else:
                        accv = _ap(acc, rl * 128 + cl, [(128, rc), (1, cc)])
                        nc.vector.scalar_tensor_tensor(
                            accv, rhs, C["wdw"][:, ct, ti:ti + 1], accv,
                            ALU.mult, ALU.add)
                zc = work.tile([128, CHUNK], BF16, tag="zc", name="zc", bufs=3)
                if pe_taps and acc is not None:
                    nc.vector.scalar_tensor_tensor(zc, pt, 1.0, acc,
                                                   ALU.mult, ALU.add)
                    nc.vector.tensor_scalar(zc, zc, C["bd"][:, ct:ct + 1], 0.0,
                                            ALU.add, ALU.max)
                    pass
                elif pe_taps:
                    nc.vector.tensor_scalar(zc, pt, C["bd"][:, ct:ct + 1], 0.0,
                                            ALU.add, ALU.max)
                else:
                    nc.vector.tensor_scalar(zc, acc, C["bd"][:, ct:ct + 1], 0.0,
                                            ALU.add, ALU.max)
                zts.append(zc)
            if DEBUG:
                tmpz = work.tile([128, 512], F32, tag="dbg", name="dbg")
                nc.vector.tensor_copy(tmpz, zts[0])
                nc.sync.dma_start(out=T["d_z0"][:, c * CHUNK:(c + 1) * CHUNK],
                                  in_=tmpz)
            pt = mmtile()
            for kt in range(4):
                nc.tensor.matmul(pt, C["wpT"][:, kt, :], zts[kt],
                                 start=(kt == 0), stop=(kt == 3))
            t6 = work.tile([128, CHUNK], F32, tag="qkv6", name="qkv6")
            nc.scalar.activation(t6, pt, AF.Identity, bias=C["bp6"],
                                 scale=1.0 / 6.0)
            nc.sync.dma_start(out=qkv32_dram[:, c * CHUNK:(c + 1) * CHUNK],
                              in_=t6)
            if DEBUG:
                nc.sync.dma_start(out=T["d_qkv"][:, c * CHUNK:(c + 1) * CHUNK],
                                  in_=t6)

    out = T["out"]

    def emit_stage7_m2(m2):
        xr3, xc3 = S7["xr3"], S7["xc3"]
        for half in range(2):
            xxt = []
            for ht in range(2):
                xr_src = bass.AP(
                    tensor=xr3[ht].tensor,
                    offset=xr3[ht].offset + m2 * 32 + half * 16,
                    ap=[list(xr3[ht].ap[0]), [4, 4], [1, 4], [0, 32]])
                xc_src = bass.AP(
                    tensor=xc3[ht].tensor, offset=xc3[ht].offset + m2 * 32,
                    ap=[list(xc3[ht].ap[0]), [0, 4], [0, 4], [1, 32]])
                t = work.tile([128, 512], BF16, tag="xxa", name="xxa")
                nc.gpsimd.tensor_add(t, xr_src, xc_src)
                xx = work.tile([128, 512], BF16, tag="xxc", name="xxc")
                nc.vector.scalar_tensor_tensor(
                    xx, t, 1.0,
                    vts[ht][:, m2 * 1024 + half * 512:
                            m2 * 1024 + (half + 1) * 512],
                    ALU.mult, ALU.add)
                nc.scalar.activation(xx, xx, AF.Relu)
                xxt.append(xx)
            sl_out = slice(m2 * 1024 + half * 512, m2 * 1024 + (half + 1) * 512)
            pt = mmtile()
            for kt in range(2):
                nc.tensor.matmul(pt, C["woT"][:, kt, :], xxt[kt],
                                 start=(kt == 0), stop=(kt == 1))
            tat = work.tile([128, 512], F32, tag="qkv6", name="attT")
            nc.scalar.activation(tat, pt, AF.Relu, bias=C["bo3"])
            qv = work.tile([128, 512], F32, tag="qvback", name="qvback")
            nc.sync.dma_start(out=qv, in_=qkv32_dram[:, sl_out])
            ot = work.tile([128, 512], F32, tag="outT", name="outT")
            nc.vector.scalar_tensor_tensor(ot, tat, 6.0, qv, ALU.min, ALU.mult)
            nc.sync.dma_start(out=out[:, sl_out], in_=ot)

    # interleaved emission schedule
    S7_SCHED = {4: range(7, 9), 5: range(9, 11), 6: range(11, 13),
                7: range(13, 15)}
    for b in range(8):
        emit_band(b)
        for _ in range(4):
            if att_queue:
                d, h = att_queue.pop(0)
                sx_, svT_, pd_, tg_, xr_ = att_units[d]
                attention_unit(sx_, svT_, pd_, tg_, xr_, h)
        if b == 3:
            S7["xr3"] = post_conv(xrr, C["wrT"], C["br"], "r")
            S7["xc3"] = post_conv(xcr, C["wc2T"], C["bc2"], "c")
            if DEBUG:
                for m in range(2):
                    tmpx = work.tile([128, 512], F32, tag="dbg", name="dbg")
                    nc.vector.tensor_copy(tmpx, S7["xr3"][m])
                    nc.sync.dma_start(out=T["d_xr3"][:, m * 512:(m + 1) * 512],
                                      in_=tmpx)
            for m2_ in range(0, 7):
                emit_stage7_m2(m2_)
        for m2 in S7_SCHED.get(b, []):
            emit_stage7_m2(m2)
    for m2 in range(15, 16):
        emit_stage7_m2(m2)

    for p in reversed(ctxs):
        p.__exit__(None, None, None)


# ======================= host side =======================

_NC_CACHE = {}


def _prep_shared(d):
    IM = _interp_mat(16, 128)
    sh = {}
    wcc = _f32(d["wcc"]) * _f32(d["gcc"])[:, None, None, None]
    wccp = np.zeros((128, 3, 128), np.float32)
    wccs = np.zeros((64, 3, 128), np.float32)
    for di in range(3):
        wccp[0:64, di, :] = wcc[:, :, di, 0].T
        wccp[64:128, di, :] = wcc[:, :, di, 1].T
        wccs[:, di, :] = wcc[:, :, di, 2].T
    sh["wccp"] = _bf(wccp)
    sh["wccs"] = _bf(wccs)
    sh["bcc"] = _f32(d["bcc"])[:, None]
    wen = _f32(d["wen"])[:, :, 0, 0] * _f32(d["gen"])[:, None]
    wen_h = wen.astype(ml_dtypes.bfloat16).astype(np.float32)
    sh["wenT_h"] = _bf(wen_h.T)
    sh["wenT_l"] = _bf((wen - wen_h).T)
    sh["ben"] = _f32(d["ben"])[:, None]
    sh["ident_b"] = _bf(np.eye(128))
    sh["ident_f"] = _f32(np.eye(128))
    wq = _f32(d["wq"])[:, :, 0, 0] * _f32(d["gq"])[:, None]
    wk = _f32(d["wk"])[:, :, 0, 0] * _f32(d["gk"])[:, None]
    wv = _f32(d["wv"])[:, :, 0, 0] * _f32(d["gv"])[:, None]
    sh["wqT"] = _bf(wq.T)
    sh["wkT"] = _bf(wk.T)
    sh["wvT"] = _bf(wv.T)
    sh["bq"] = _f32(d["bq"])[:, None]
    sh["bk"] = _f32(d["bk"])[:, None]
    sh["bv"] = _f32(d["bv"])[:128, None]
    sh["bv2"] = _f32(d["bv"])[128:, None]
    sh["wqsT"] = _bf(wq.T * (SCALE / 32.0))
    sh["wksT"] = _bf(wk.T / 32.0)
    sh["wvsT"] = _bf(wv.T / 32.0)
    sh["bq2"] = _f32((_f32(d["bq"]) * SCALE).reshape(8, 16).T)
    sh["bk2"] = _f32(_f32(d["bk"]).reshape(8, 16).T)
    for nm, kq, kk in (("posqk_r", "prq", "prk"), ("posqk_c", "pcq", "pck")):
        pq = (_f32(d[kq]) @ IM.T).reshape(4, 8, 16, 128).transpose(2, 1, 0, 3)
        pk = (_f32(d[kk]) @ IM.T).reshape(4, 8, 16, 128).transpose(2, 1, 0, 3)
        both = np.zeros((16, 8, 2, 512), np.float32)
        both[:, :, 0, :] = pq.reshape(16, 8, 512) * SCALE
        both[:, :, 1, :] = pk.reshape(16, 8, 512)
        sh[nm] = _bf(both.reshape(16, 8192))
    sh["bvT"] = _bf(np.broadcast_to(_f32(d["bv"])[None, :], (128, 256)))
    sh["ones1"] = _bf(np.ones((128, 1)))
    sh["ones2"] = _f32(np.ones((1, 128)))
    wd = _f32(d["wd"])[:, 0] * _f32(d["gd"])[:, None, None]
    wd9 = wd.reshape(512, 9)
    order = [4, 0, 1, 2, 3, 5, 6, 7, 8]
    wd9o = wd9[:, order]
    sh["wdw"] = _f32(wd9o.reshape(4, 128, 9).transpose(1, 0, 2))
    wdiag = np.zeros((4, 9, 128, 128), np.float32)
    for ct in range(4):
        for t in range(9):
            np.fill_diagonal(wdiag[ct, t], wd9o[ct * 128:(ct + 1) * 128, t])
    sh["wdiag"] = _bf(wdiag.transpose(2, 0, 1, 3))
    sh["bd"] = _f32(_f32(d["bd"]).reshape(4, 128).T)
    wp = _f32(d["wp"])[:, :, 0, 0] * _f32(d["gp"])[:, None]
    sh["wpT"] = _bf(wp.T.reshape(4, 128, 128).transpose(1, 0, 2))
    sh["bp6"] = _f32(d["bp"])[:, None] / 6.0
    wr = _f32(d["wr"])[:, :, 0, 0] * _f32(d["gr"])[:, None]
    sh["wrT"] = _bf(wr.T.reshape(2, 128, 256).transpose(1, 0, 2))
    sh["br"] = _f32(_f32(d["br"]).reshape(2, 128).T)
    wc2 = _f32(d["wc2"])[:, :, 0, 0] * _f32(d["gc2"])[:, None]
    sh["wc2T"] = _bf(wc2.T.reshape(2, 128, 256).transpose(1, 0, 2))
    sh["bc2"] = _f32(_f32(d["bc2"]).reshape(2, 128).T)
    wo = _f32(d["wo"])[:, :, 0, 0] * _f32(d["go"])[:, None]
    sh["woT"] = _bf(wo.T.reshape(2, 128, 128).transpose(1, 0, 2))
    sh["bo3"] = _f32(d["bo"])[:, None] + 3.0
    return sh


def _prep_core(x_b):
    xp = np.zeros((64, H + 2, WP), np.float32)
    xp[:, 1:129, 1:129] = x_b
    flat = xp.reshape(64, NPAD)
    shift = np.zeros_like(flat)
    shift[:, :-1] = flat[:, 1:]
    return _bf(np.concatenate([flat, shift], axis=0))


def kernel(**inputs):
    from concourse.bass_utils import run_bass_kernel_spmd
    sh = _prep_shared(inputs)
    nc = _NC_CACHE.get("nc")
    if nc is None:
        nc = build_nc()
        _NC_CACHE["nc"] = nc
    x = _f32(inputs["x"])
    in_maps = []
    for b in range(8):
        m = dict(sh)
        m["xstack"] = _prep_core(x[b])
        in_maps.append(m)
    res = run_bass_kernel_spmd(nc, in_maps, core_ids=list(range(8)),
                               trace=bool(int(os.environ.get("KTRACE", "0"))))
    if res.exec_time_ns is not None:
        print(f"HW exec time: {res.exec_time_ns} ns")
    _NC_CACHE["last"] = res
    out = np.stack([r["out"].reshape(128, 128, 128) for r in res.results])
    return out.astype(np.float32)


if __name__ == "__main__":
    build_nc()
    print("built ok")



# revision 26
# speedup vs baseline: 1.0788x; 1.0788x over previous
"""Trainium2 Bass kernel for nn_ASCNet (sparse axial attention net).

Data-parallel over batch: 8 NeuronCores, 1 image each. Full inputs in,
full output out. Heavy matmuls in bf16 with fp32 PSUM accumulation; the
CCAM `e` path is kept fp32-accurate via hi/lo bf16 splits because its
softmax is near-argmin and tie-sensitive.

The qkv 1x1 projections are fused into the depthwise 3x3 taps: one PE
matmul per (out-block, tap) with shifted/masked views of xs, with the
projection biases restored exactly via per-edge PSUM fixups. qkv stays
resident in SBUF (no DRAM roundtrip).

Layout: activations as [channel partitions, spatial free], spatial
row-major (n = h*128 + w).
"""

import os
import sys

sys.path.insert(0, "/opt/trn_rl_repo")

import numpy as np
import ml_dtypes

import concourse.bass as bass
import concourse.mybir as mybir
import concourse.tile as tile

BF16 = mybir.dt.bfloat16
F32 = mybir.dt.float32
AF = mybir.ActivationFunctionType
ALU = mybir.AluOpType
AX = mybir.AxisListType

H = W = 128
N = H * W
WP = W + 2
NPAD = (H + 2) * WP
CHUNK = 512
NCHUNK = N // CHUNK
SCALE = 16 ** -0.5

DEBUG = bool(int(os.environ.get("KDEBUG", "0")))


# ---- walrus workaround: split multi-wait instructions into sem-wait NOPs ----
import orjson as _orjson


def _split_waits_json(bir_bytes):
    d = _orjson.loads(bir_bytes)
    uid = [0]
    for f in d.get("functions", []):
        for bb in f.get("blocks", []):
            insts = bb.get("instructions", [])
            out = []
            for ins in insts:
                si = ins.get("sync_info") or {}
                waits = si.get("on_wait") or []
                if len(waits) > 1:
                    eng = ins.get("engine")
                    dbg = ins.get("debug")
                    for wv in waits[:-1]:
                        uid[0] += 1
                        nop = {
                            "engine": eng,
                            "ins": [],
                            "outs": [],
                            "name": f"wsplit_{uid[0]}",
                            "opcode": "EventSemaphore",
                            "sync_info": {"on_update": [], "on_wait": [wv]},
                        }
                        if dbg is not None:
                            nop["debug"] = dbg
                        out.append(nop)
                    si["on_wait"] = [waits[-1]]
                    ins["sync_info"] = si
                out.append(ins)
            bb["instructions"] = out
    return _orjson.dumps(d)


_orig_mtjb = mybir.module_to_json_bytes


def _patched_mtjb(m):
    return _split_waits_json(_orig_mtjb(m))


mybir.module_to_json_bytes = _patched_mtjb

TAPS = [(0, 0), (-1, -1), (-1, 0), (-1, 1), (0, -1), (0, 1),
        (1, -1), (1, 0), (1, 1)]


def _bf(a):
    return np.ascontiguousarray(np.asarray(a)).astype(ml_dtypes.bfloat16)


def _f32(a):
    return np.ascontiguousarray(np.asarray(a)).astype(np.float32)


def _interp_mat(n_in, n_out):
    M = np.zeros((n_out, n_in), np.float64)
    scale = n_in / n_out
    for i in range(n_out):
        c = (i + 0.5) * scale - 0.5
        lo = int(np.floor(c))
        frac = c - lo
        M[i, min(max(lo, 0), n_in - 1)] += 1 - frac
        M[i, min(max(lo + 1, 0), n_in - 1)] += frac
    return M.astype(np.float32)


def _ap(t, off, dims, parts=None):
    p0 = list(t.ap[0])
    if parts is not None:
        p0 = [p0[0], parts]
    return bass.AP(tensor=t.tensor, offset=t.offset + off,
                   ap=[p0] + [[s, c] for (s, c) in dims])


def build_nc():
    nc = bass.Bass()
    T = {}

    def din(name, shape, dtype):
        t = nc.dram_tensor(name, list(shape), dtype, kind="ExternalInput")
        T[name] = t[:]

    din("xstack", (128, NPAD), BF16)
    din("wccp", (128, 3, 128), BF16)
    din("wccs", (64, 3, 128), BF16)
    din("bcc", (128, 1), F32)
    din("wenT_h", (128, 8), BF16)
    din("wenT_l", (128, 8), BF16)
    din("ben", (8, 1), F32)
    din("ident_b", (128, 128), BF16)
    din("ident_f", (128, 128), F32)
    din("wvT", (128, 256), BF16)
    din("bv", (128, 1), F32)
    din("bv2", (128, 1), F32)
    din("wqsT", (128, 128), BF16)
    din("wksT", (128, 128), BF16)
    din("wvsT", (128, 256), BF16)
    din("bq2", (16, 8), F32)
    din("bk2", (16, 8), F32)
    din("posqk_r", (16, 8192), BF16)   # cols (h, q/k, j, L)
    din("posqk_c", (16, 8192), BF16)
    din("bvT", (128, 256), BF16)
    din("ones1", (128, 1), BF16)
    din("ones2", (1, 128), BF16)
    din("wfuse", (128, 4, 9, 128), BF16)
    din("dwbias", (128, 4, 1), F32)   # interior bias (bd + b*sum(wd))
    din("cwed", (2, 4, 128), BF16)    # w0 / w127 column corrections
    din("chrow0", (1, 4, 128), BF16)  # h0 row corrections
    din("chrow1", (1, 4, 128), BF16)  # h127 row corrections
    din("corn0", (2, 4, 128), BF16)   # c00, c0W corner fixes
    din("corn1", (2, 4, 128), BF16)   # cH0, cHW corner fixes
    din("indw", (2, 8), BF16)         # edge-parity indicator rows
    din("ind2", (2, 2), BF16)
    din("onesr", (1, 128), BF16)
    din("wpT", (128, 4, 128), BF16)
    din("bp6", (128, 1), F32)
    din("wrT", (128, 2, 256), BF16)
    din("br", (128, 2), F32)
    din("wc2T", (128, 2, 256), BF16)
    din("bc2", (128, 2), F32)
    din("woT", (128, 2, 128), BF16)
    din("bo3", (128, 1), F32)

    T["out"] = nc.dram_tensor("out", [128, N], F32, kind="ExternalOutput")[:]
    if DEBUG:
        for nm, shp in (("d_cb", (128, N)), ("d_xs", (128, N)),
                        ("d_z0", (128, N)), ("d_qkv", (128, N)),
                        ("d_e", (128, 8)), ("d_sxr", (128, 512)),
                        ("d_xr3", (128, 1024)), ("d_vf", (128, N))):
            T[nm] = nc.dram_tensor(nm, list(shp), F32, kind="ExternalOutput")[:]

    with tile.TileContext(nc) as tc:
        _emit(nc, tc, T)
    return nc


def _emit(nc, tc, T):
    ctxs = []

    def pool(name, bufs, space="SBUF"):
        p = tc.tile_pool(name=name, bufs=bufs, space=space)
        ctxs.append(p)
        return p.__enter__()

    const = pool("const", 1)
    big = pool("big", 1)
    small = pool("small", 1)
    work = pool("work", 2)
    psDW = pool("psDW", 3, "PSUM")
    psMM = pool("psMM", 2, "PSUM")
    psB = pool("psB", 2, "PSUM")
    psC = pool("psC", 1, "PSUM")

    def mmtile(shape=(128, CHUNK), dt=F32):
        return psMM.tile(list(shape), dt, tag="mm", name="mm")

    C = {}
    for name in ("wccp", "wccs", "bcc", "wenT_h", "wenT_l", "ben",
                 "ident_b", "ident_f", "wvT", "bv", "bv2",
                 "wqsT", "wksT", "wvsT", "bq2", "bk2", "bvT", "ones1",
                 "ones2", "wfuse", "dwbias", "cwed", "chrow0", "chrow1",
                 "corn0", "corn1", "indw", "ind2", "onesr", "wpT", "bp6",
                 "wrT", "br", "wc2T", "bc2", "woT", "bo3"):
        src = T[name]
        t = const.tile(list(src.shape), src.dtype, tag=name, name=name)
        nc.gpsimd.dma_start(out=t, in_=src)
        C[name] = t

    # big slots (tag -> lifetimes):
    #  bigA: xstack(s1) -> cf16(s2-3) -> qkv(s6+)
    #  bigE: xstack2(s1)
    #  bigB: cb(s1-3)   -> vf0(s6-8)
    #  bigC: cbT(s3)    -> vf1(s6-8)
    #  bigD: xs(s3-6)
    xstack = big.tile([128, NPAD], BF16, tag="bigA", name="xstack")
    nc.sync.dma_start(out=xstack, in_=T["xstack"])

    def dump(name, src_tile, ncols):
        if not DEBUG:
            return
        for c0 in range(0, ncols, 512):
            w = min(512, ncols - c0)
            tmp = work.tile([128, w], F32, tag="dbg", name="dbg")
            nc.vector.tensor_copy(tmp, src_tile[:, c0:c0 + w])
            nc.sync.dma_start(out=T[name][:, c0:c0 + w], in_=tmp)

    # ======== stage 1: conv3x3 64->128 -> cb (bf16); cbT via xbar DMA ====
    cb = big.tile([128, N], BF16, tag="bigB", name="cb")
    cbT = big.tile([128, N], BF16, tag="bigC", name="cbT")
    cbT3 = cbT.rearrange("p (t j) -> p t j", j=128)
    for c in range(NCHUNK):
        h0 = c * 4
        pt = psDW.tile([128, CHUNK], F32, tag="dw", name="s1")
        for di in range(3):
            rhs = _ap(xstack, (h0 + di) * WP, [(WP, 4), (1, 128)])
            nc.tensor.matmul(pt, C["wccp"][:, di, :], rhs,
                             start=(di == 0), stop=False)
        for di in range(3):
            rhs = _ap(xstack, (h0 + di) * WP + 2, [(WP, 4), (1, 128)], parts=64)
            nc.tensor.matmul(pt, C["wccs"][:, di, :], rhs,
                             start=False, stop=(di == 2))
        nc.scalar.activation(cb[:, c * CHUNK:(c + 1) * CHUNK], pt, AF.Relu,
                             bias=C["bcc"])
        if c % 8 == 7:
            k = c // 8
            eng = nc.sync if k % 2 == 0 else nc.scalar
            eng.dma_start_transpose(
                cbT3[:, k * 32:(k + 1) * 32, :],
                cb[:, k * 4096:(k + 1) * 4096])
    dump("d_cb", cb, N)

    # ======== stage 2: cf = relu(wen.cb) (hi/lo); cf16 kept, cfT built ====
    cf16 = big.tile([8, N], BF16, tag="bigA", name="cf16")
    cfT = small.tile([128, 128, 16], BF16, tag="cfT", name="cfT")
    for g in range(2):
        ptT = psB.tile([128, 512], F32, tag="tp", name="tpf")
        for cc_ in range(16):
            c = g * 16 + cc_
            pc = psC.tile([8, CHUNK], F32, tag="oc", name="oc")
            nc.tensor.matmul(pc, C["wenT_h"], cb[:, c * CHUNK:(c + 1) * CHUNK],
                             start=True, stop=False)
            nc.tensor.matmul(pc, C["wenT_l"], cb[:, c * CHUNK:(c + 1) * CHUNK],
                             start=False, stop=True)
            cf32c = work.tile([8, CHUNK], F32, tag="qkv6", name="cf32c", bufs=1)
            nc.scalar.activation(cf32c, pc, AF.Relu, bias=C["ben"])
            nc.vector.tensor_copy(cf16[:, c * CHUNK:(c + 1) * CHUNK], cf32c)
            for i in range(4):
                nc.tensor.transpose(ptT[:, (cc_ * 4 + i) * 8:(cc_ * 4 + i + 1) * 8],
                                    cf32c[:, i * 128:(i + 1) * 128],
                                    C["ident_f"][0:8, 0:8])
        # ptT holds cf32.T for 64 tiles: write hi into cfT[...,0:8],
        # lo residual into cfT[...,8:16]
        tdst_h = _ap(cfT, g * 64 * 16, [(16, 64), (1, 8)])
        tdst_l = _ap(cfT, g * 64 * 16 + 8, [(16, 64), (1, 8)])
        nc.scalar.activation(tdst_h, ptT, AF.Copy)
        nc.vector.scalar_tensor_tensor(tdst_l, ptT, 1.0, tdst_h,
                                       ALU.mult, ALU.subtract)

    # ======== stage 3: e, softmax, dec, xs ========
    pe2 = psC.tile([16, 128], F32, tag="oc", name="pe2")
    for t in range(N // 128):
        nc.tensor.matmul(pe2, cfT[:, t, :], cbT[:, t * 128:(t + 1) * 128],
                         start=(t == 0), stop=(t == N // 128 - 1))
    e2sb = work.tile([16, 128], F32, tag="e2sb", name="e2sb", bufs=1)
    nc.scalar.activation(e2sb, pe2, AF.Copy)
    peT = psB.tile([128, 16], F32, tag="tp", name="peT")
    nc.tensor.transpose(peT, e2sb, C["ident_f"][0:16, 0:16])
    peTs = work.tile([128, 16], F32, tag="peTs", name="peTs", bufs=1)
    nc.scalar.activation(peTs, peT, AF.Copy)
    e_sb = small.tile([128, 8], F32, tag="e_sb", name="e_sb")
    nc.vector.tensor_tensor(e_sb, peTs[:, 0:8], peTs[:, 8:16], ALU.add)
    if DEBUG:
        nc.sync.dma_start(out=T["d_e"], in_=e_sb)
    emin = small.tile([128, 1], F32, tag="emin", name="emin")
    nc.vector.tensor_reduce(emin, e_sb, AX.X, ALU.min)
    a_sb = small.tile([128, 8], F32, tag="a_sb", name="a_sb")
    nc.scalar.activation(a_sb, e_sb, AF.Exp, bias=emin, scale=-1.0)
    asum = small.tile([128, 1], F32, tag="asum", name="asum")
    nc.vector.tensor_reduce(asum, a_sb, AX.X, ALU.add)
    arec = small.tile([128, 1], F32, tag="arec", name="arec")
    nc.vector.reciprocal(arec, asum)
    a16 = small.tile([128, 8], BF16, tag="a16", name="a16")
    nc.vector.tensor_scalar(a16, a_sb, arec, None, ALU.mult)
    paT = psC.tile([8, 128], BF16, tag="oc", name="paT")
    nc.tensor.transpose(paT, a16, C["ident_b"])
    aT = small.tile([8, 128], BF16, tag="aT_sb", name="aT_sb")
    nc.scalar.activation(aT, paT, AF.Copy)

    xs = big.tile([128, N], BF16, tag="bigD", name="xs")
    sxr32 = work.tile([128, 4, 128], F32, tag="sx32r", name="sxr32", bufs=1)
    sxc32 = work.tile([128, 4, 128], F32, tag="sx32c", name="sxc32", bufs=1)
    for c in range(NCHUNK):
        sl = slice(c * CHUNK, (c + 1) * CHUNK)
        pt = mmtile()
        nc.tensor.matmul(pt, aT, cf16[:, sl], start=True, stop=True)
        nc.vector.scalar_tensor_tensor(xs[:, sl], pt, 0.5, cb[:, sl],
                                       ALU.mult, ALU.add)
        if c % 8 == 7:
            qv_ = c // 8
            # sxr partial: h-range [32q, 32q+32): sum over w-groups
            src_ap = bass.AP(tensor=xs.tensor, offset=xs.offset + qv_ * 4096,
                             ap=[list(xs.ap[0]), [1, 4], [128, 32], [4, 32]])
            dst_ap = bass.AP(tensor=sxr32.tensor,
                             offset=sxr32.offset + qv_ * 32,
                             ap=[list(sxr32.ap[0]), [128, 4], [1, 32]])
            nc.vector.tensor_reduce(dst_ap, src_ap, AX.X, ALU.add)
            # sxc partial: group g = qv_ sums 32 contiguous h rows
            src_c = bass.AP(tensor=xs.tensor, offset=xs.offset + qv_ * 4096,
                            ap=[list(xs.ap[0]), [1, 128], [128, 32]])
            dst_c = bass.AP(tensor=sxc32.tensor,
                            offset=sxc32.offset + qv_ * 128,
                            ap=[list(sxc32.ap[0]), [1, 128]])
            nc.vector.tensor_reduce(dst_c, src_c, AX.X, ALU.add)
    dump("d_xs", xs, N)

    # ======== stage 4: finish shunts + shunt-v (svT) ========
    sxr = small.tile([128, 512], BF16, tag="sxr", name="sxr")
    nc.vector.tensor_copy(sxr, sxr32.rearrange("p a b -> p (a b)"))
    sxc = small.tile([128, 512], BF16, tag="sxc", name="sxc")
    nc.vector.tensor_copy(sxc, sxc32.rearrange("p a b -> p (a b)"))
    dump("d_sxr", sxr, 512)

    def shunt_v(sx, tagp):
        svT = small.tile([128, 4, 256], BF16, tag=f"svT{tagp}", name="svT")
        for j in range(4):
            pv = mmtile((128, 256))
            nc.tensor.matmul(pv, sx[:, j * 128:(j + 1) * 128], C["wvsT"],
                             start=True, stop=True)
            nc.vector.scalar_tensor_tensor(svT[:, j, :], pv, 1.0, C["bvT"],
                                           ALU.mult, ALU.add)
        return svT

    svT_r = shunt_v(sxr, "r")
    svT_c = shunt_v(sxc, "c")

    # ======== stages 5/6/7 interleaved ========
    def attention_unit(sx, svT, posdram, tagp, xr_relu, h):
        post = work.tile([16, 1024], BF16, tag="post", name="post", bufs=1)
        nc.sync.dma_start(out=post, in_=posdram[:, h * 1024:(h + 1) * 1024])
        qk_h = work.tile([16, 1024], BF16, tag="qkh", name="qkh", bufs=1)
        pq2 = psC.tile([16, 512], F32, tag="oc", name="pq2")
        nc.tensor.matmul(pq2, C["wqsT"][:, h * 16:(h + 1) * 16], sx,
                         start=True, stop=True)
        nc.vector.scalar_tensor_tensor(qk_h[:, 0:512], pq2,
                                       C["bq2"][:, h:h + 1],
                                       post[:, 0:512], ALU.add, ALU.add)
        pk2 = psC.tile([16, 512], F32, tag="oc", name="pk2")
        nc.tensor.matmul(pk2, C["wksT"][:, h * 16:(h + 1) * 16], sx,
                         start=True, stop=True)
        nc.vector.scalar_tensor_tensor(qk_h[:, 512:1024], pk2,
                                       C["bk2"][:, h:h + 1],
                                       post[:, 512:1024], ALU.add, ALU.add)
        ht, hr = divmod(h, 4)
        expT4 = work.tile([128, 512], BF16, tag="expT4", name="expT4", bufs=1)
        for j in range(4):
            pl = psB.tile([128, 128], F32, tag="tp", name="pl")
            nc.tensor.matmul(pl, qk_h[:, 512 + j * 128: 512 + (j + 1) * 128],
                             qk_h[:, j * 128:(j + 1) * 128],
                             start=True, stop=True)
            nc.scalar.activation(expT4[:, j * 128:(j + 1) * 128], pl, AF.Exp)
        pz = psC.tile([1, 512], F32, tag="oc", name="pz")
        nc.tensor.matmul(pz, C["ones1"], expT4, start=True, stop=True)
        zrec = work.tile([1, 512], BF16, tag="zrec", name="zrec", bufs=2)
        with nc.allow_low_precision(reason="attn z normalizer in bf16"):
            nc.vector.reciprocal(zrec, pz)
        pzb = psB.tile([128, 512], F32, tag="tp", name="pzb")
        nc.tensor.matmul(pzb, C["ones2"], zrec, start=True, stop=True)
        pzs = work.tile([128, 512], BF16, tag="pzs", name="pzs", bufs=1)
        nc.scalar.activation(pzs, pzb, AF.Copy)
        tno = work.tile([128, 128], BF16, tag="tno", name="tno", bufs=2)
        for j in range(4):
            po = psC.tile([128, 128], F32, tag="oc", name="po")
            nc.tensor.matmul(po[hr * 32:(hr + 1) * 32, :],
                             svT[:, j, h * 32:(h + 1) * 32],
                             expT4[:, j * 128:(j + 1) * 128],
                             start=True, stop=True,
                             tile_position=(0, hr * 32))
            nc.vector.tensor_mul(tno[hr * 32:(hr + 1) * 32, :],
                                 po[hr * 32:(hr + 1) * 32, :],
                                 pzs[hr * 32:(hr + 1) * 32,
                                     j * 128:(j + 1) * 128])
            nc.scalar.activation(
                xr_relu[ht][hr * 32:(hr + 1) * 32, j * 128:(j + 1) * 128],
                tno[hr * 32:(hr + 1) * 32, :], AF.Relu)

    xrr = [small.tile([128, 512], BF16, tag=f"xrer{i}", name="xre")
           for i in range(2)]
    xcr = [small.tile([128, 512], BF16, tag=f"xrec{i}", name="xre")
           for i in range(2)]
    att_units = [(sxr, svT_r, T["posqk_r"], "r", xrr),
                 (sxc, svT_c, T["posqk_c"], "c", xcr)]
    att_queue = [(d, h) for h in range(8) for d in range(2)]

    def post_conv(xin, wT, bias_col, tagp):
        out_t = [small.tile([128, 512], BF16, tag=f"x3{tagp}{m}", name="x3")
                 for m in range(2)]
        for m in range(2):
            pt = mmtile()
            for kt in range(2):
                nc.tensor.matmul(pt, wT[:, kt, m * 128:(m + 1) * 128],
                                 xin[kt], start=(kt == 0), stop=(kt == 1))
            nc.scalar.activation(out_t[m], pt, AF.Identity,
                                 bias=bias_col[:, m:m + 1])
        return out_t

    # stage-6 state
    qkv = big.tile([128, N], BF16, tag="bigA", name="qkv")
    vf0 = big.tile([128, N], BF16, tag="bigB", name="vf0")
    vf1 = big.tile([128, N], BF16, tag="bigC", name="vf1")
    vts = (vf0, vf1)
    zc_t = [small.tile([128, 4, 512], BF16, tag=f"zc{ct}", name="zc")
            for ct in range(4)]

    def tap_ranges(c, di, dj):
        h0 = c * 4
        r_lo, r_hi = 0, 4
        if h0 + di < 0:
            r_lo = -di - h0
        if h0 + 3 + di > 127:
            r_hi = 4 - (h0 + 3 + di - 127)
        c_lo = max(0, -dj)
        c_hi = min(128, 128 - dj)
        return r_lo, r_hi - r_lo, c_lo, c_hi - c_lo

    S7 = {}

    def emit_band(b):
        # v projections straight into vf0/vf1 (needed for stage 7)
        for vt in range(2):
            for q in range(4):
                c = 4 * b + q
                sl = slice(c * CHUNK, (c + 1) * CHUNK)
                pv = mmtile()
                nc.tensor.matmul(pv, C["wvT"][:, vt * 128:(vt + 1) * 128],
                                 xs[:, sl], start=True, stop=True)
                if (vt + q) % 2 == 0:
                    nc.scalar.activation(vts[vt][:, sl], pv, AF.Identity,
                                         bias=C["bv" if vt == 0 else "bv2"])
                else:
                    nc.vector.tensor_scalar(
                        vts[vt][:, sl], pv,
                        C["bv" if vt == 0 else "bv2"], None, ALU.add)
        # fused qkv-projection + depthwise taps
        for ct in range(4):
            for qp in range(2):           # chunk pairs -> 2 psDW tiles live
                pts = []
                for qq in range(2):
                    q = qp * 2 + qq
                    c = 4 * b + q
                    pt = psDW.tile([128, CHUNK], F32, tag="dw", name="dw")
                    pts.append((q, c, pt))
                for ti in range(9):
                    di, dj = TAPS[ti]
                    for (q, c, pt) in pts:
                        rl, rc, cl, cc = tap_ranges(c, di, dj)
                        off = (4 * c + rl + di) * 128 + cl + dj
                        rhs = _ap(xs, off, [(128, rc), (1, cc)])
                        outp = _ap(pt, rl * 128 + cl, [(128, rc), (1, cc)])
                        nc.tensor.matmul(outp, C["wfuse"][:, ct, ti, :], rhs,
                                         start=(ti == 0), stop=False,
                                         skip_group_check=True)
                for (q, c, pt) in pts:
                    # exact projection-bias edge fixups, accumulated on PE:
                    # tiny indicator matmuls finish the accumulation group
                    fixes = [(_ap(pt, 0, [(128, 4), (127, 2)]),
                              C["cwed"][:, ct, :], C["indw"][:])]
                    if c == 0:
                        fixes.append((_ap(pt, 0, [(1, 128)]),
                                      C["chrow0"][:, ct, :], C["onesr"][:]))
                        fixes.append((_ap(pt, 0, [(127, 2)]),
                                      C["corn0"][:, ct, :], C["ind2"][:]))
                    if c == 31:
                        fixes.append((_ap(pt, 384, [(1, 128)]),
                                      C["chrow1"][:, ct, :], C["onesr"][:]))
                        fixes.append((_ap(pt, 384, [(127, 2)]),
                                      C["corn1"][:, ct, :], C["ind2"][:]))
                    for fi, (outp, lhsT, rhs) in enumerate(fixes):
                        nc.tensor.matmul(outp, lhsT, rhs, start=False,
                                         stop=(fi == len(fixes) - 1),
                                         skip_group_check=True)
                    # bias + relu -> zc (alternate scalar/vector)
                    if (ct + q) % 2 == 0:
                        nc.scalar.activation(zc_t[ct][:, q, :], pt, AF.Relu,
                                             bias=C["dwbias"][:, ct, 0:1])
                    else:
                        nc.vector.tensor_scalar(zc_t[ct][:, q, :], pt,
                                                C["dwbias"][:, ct, 0:1], 0.0,
                                                ALU.add, ALU.max)
        if DEBUG:
            tmpz = work.tile([128, 512], F32, tag="dbg", name="dbg")
            nc.vector.tensor_copy(tmpz, zc_t[0][:, 0, :])
            nc.sync.dma_start(out=T["d_z0"][:, (4 * b) * CHUNK:
                                            (4 * b + 1) * CHUNK], in_=tmpz)
        # wp: 512 -> 128, write qkv/6 into SBUF-resident qkv (bf16)
        for q in range(4):
            c = 4 * b + q
            pt = mmtile()
            for kt in range(4):
                nc.tensor.matmul(pt, C["wpT"][:, kt, :], zc_t[kt][:, q, :],
                                 start=(kt == 0), stop=(kt == 3))
            nc.scalar.activation(qkv[:, c * CHUNK:(c + 1) * CHUNK], pt,
                                 AF.Identity, bias=C["bp6"], scale=1.0 / 6.0)
        if DEBUG:
            for q in range(4):
                c = 4 * b + q
                tmpq = work.tile([128, 512], F32, tag="dbg", name="dbg")
                nc.vector.tensor_copy(tmpq, qkv[:, c * CHUNK:(c + 1) * CHUNK])
                nc.sync.dma_start(out=T["d_qkv"][:, c * CHUNK:(c + 1) * CHUNK],
                                  in_=tmpq)

    out = T["out"]

    def emit_stage7_m2(m2):
        xr3, xc3 = S7["xr3"], S7["xc3"]
        for half in range(2):
            xxt = []
            for ht in range(2):
                xr_src = bass.AP(
                    tensor=xr3[ht].tensor,
                    offset=xr3[ht].offset + m2 * 32 + half * 16,
                    ap=[list(xr3[ht].ap[0]), [4, 4], [1, 4], [0, 32]])
                xc_src = bass.AP(
                    tensor=xc3[ht].tensor, offset=xc3[ht].offset + m2 * 32,
                    ap=[list(xc3[ht].ap[0]), [0, 4], [0, 4], [1, 32]])
                t = work.tile([128, 512], BF16, tag="xxa", name="xxa", bufs=1)
                nc.gpsimd.tensor_add(t, xr_src, xc_src)
                xx = work.tile([128, 512], BF16, tag="xxc", name="xxc")
                nc.vector.scalar_tensor_tensor(
                    xx, t, 1.0,
                    vts[ht][:, m2 * 1024 + half * 512:
                            m2 * 1024 + (half + 1) * 512],
                    ALU.mult, ALU.add)
                nc.scalar.activation(xx, xx, AF.Relu)
                xxt.append(xx)
            sl_out = slice(m2 * 1024 + half * 512, m2 * 1024 + (half + 1) * 512)
            pt = mmtile()
            for kt in range(2):
                nc.tensor.matmul(pt, C["woT"][:, kt, :], xxt[kt],
                                 start=(kt == 0), stop=(kt == 1))
            tat = work.tile([128, 512], F32, tag="qkv6", name="attT", bufs=1)
            nc.scalar.activation(tat, pt, AF.Relu, bias=C["bo3"])
            ot = work.tile([128, 512], F32, tag="outT", name="outT", bufs=1)
            nc.vector.scalar_tensor_tensor(ot, tat, 6.0, qkv[:, sl_out],
                                           ALU.min, ALU.mult)
            nc.sync.dma_start(out=out[:, sl_out], in_=ot)

    # interleaved emission schedule
    S7_SCHED = {4: range(7, 9), 5: range(9, 11), 6: range(11, 13),
                7: range(13, 15)}
    for b in range(8):
        emit_band(b)
        for _ in range(4):
            if att_queue:
                d, h = att_queue.pop(0)
                sx_, svT_, pd_, tg_, xr_ = att_units[d]
                attention_unit(sx_, svT_, pd_, tg_, xr_, h)
        if b == 3:
            S7["xr3"] = post_conv(xrr, C["wrT"], C["br"], "r")
            S7["xc3"] = post_conv(xcr, C["wc2T"], C["bc2"], "c")
            if DEBUG:
                for m in range(2):
                    tmpx = work.tile([128, 512], F32, tag="dbg", name="dbg")
                    nc.vector.tensor_copy(tmpx, S7["xr3"][m])
                    nc.sync.dma_start(out=T["d_xr3"][:, m * 512:(m + 1) * 512],
                                      in_=tmpx)
            for m2_ in range(0, 7):
                emit_stage7_m2(m2_)
        for m2 in S7_SCHED.get(b, []):
            emit_stage7_m2(m2)
    for m2 in range(15, 16):
        emit_stage7_m2(m2)

    for p in reversed(ctxs):
        p.__exit__(None, None, None)


# ======================= host side =======================

_NC_CACHE = {}


def _prep_shared(d):
    IM = _interp_mat(16, 128)
    sh = {}
    wcc = _f32(d["wcc"]) * _f32(d["gcc"])[:, None, None, None]
    wccp = np.zeros((128, 3, 128), np.float32)
    wccs = np.zeros((64, 3, 128), np.float32)
    for di in range(3):
        wccp[0:64, di, :] = wcc[:, :, di, 0].T
        wccp[64:128, di, :] = wcc[:, :, di, 1].T
        wccs[:, di, :] = wcc[:, :, di, 2].T
    sh["wccp"] = _bf(wccp)
    sh["wccs"] = _bf(wccs)
    sh["bcc"] = _f32(d["bcc"])[:, None]
    wen = _f32(d["wen"])[:, :, 0, 0] * _f32(d["gen"])[:, None]
    wen_h = wen.astype(ml_dtypes.bfloat16).astype(np.float32)
    sh["wenT_h"] = _bf(wen_h.T)
    sh["wenT_l"] = _bf((wen - wen_h).T)
    sh["ben"] = _f32(d["ben"])[:, None]
    sh["ident_b"] = _bf(np.eye(128))
    sh["ident_f"] = _f32(np.eye(128))
    wq = _f32(d["wq"])[:, :, 0, 0] * _f32(d["gq"])[:, None]
    wk = _f32(d["wk"])[:, :, 0, 0] * _f32(d["gk"])[:, None]
    wv = _f32(d["wv"])[:, :, 0, 0] * _f32(d["gv"])[:, None]
    sh["wvT"] = _bf(wv.T)
    sh["bv"] = _f32(d["bv"])[:128, None]
    sh["bv2"] = _f32(d["bv"])[128:, None]
    sh["wqsT"] = _bf(wq.T * (SCALE / 32.0))
    sh["wksT"] = _bf(wk.T / 32.0)
    sh["wvsT"] = _bf(wv.T / 32.0)
    sh["bq2"] = _f32((_f32(d["bq"]) * SCALE).reshape(8, 16).T)
    sh["bk2"] = _f32(_f32(d["bk"]).reshape(8, 16).T)
    for nm, kq, kk in (("posqk_r", "prq", "prk"), ("posqk_c", "pcq", "pck")):
        pq = (_f32(d[kq]) @ IM.T).reshape(4, 8, 16, 128).transpose(2, 1, 0, 3)
        pk = (_f32(d[kk]) @ IM.T).reshape(4, 8, 16, 128).transpose(2, 1, 0, 3)
        both = np.zeros((16, 8, 2, 512), np.float32)
        both[:, :, 0, :] = pq.reshape(16, 8, 512) * SCALE
        both[:, :, 1, :] = pk.reshape(16, 8, 512)
        sh[nm] = _bf(both.reshape(16, 8192))
    sh["bvT"] = _bf(np.broadcast_to(_f32(d["bv"])[None, :], (128, 256)))
    sh["ones1"] = _bf(np.ones((128, 1)))
    sh["ones2"] = _bf(np.ones((1, 128)))
    # fused depthwise weights: W9[o, c, t] = wdw[o, t] * Weff[o, c]
    wd = _f32(d["wd"])[:, 0] * _f32(d["gd"])[:, None, None]
    wd9 = wd.reshape(512, 9)
    order = [4, 0, 1, 2, 3, 5, 6, 7, 8]   # TAPS order (center first)
    wd9o = wd9[:, order]
    Weff = np.concatenate([wq, wk, wv], axis=0)       # [512, 128]
    bqkv = np.concatenate([_f32(d["bq"]), _f32(d["bk"]), _f32(d["bv"])])
    wfuse = np.zeros((128, 4, 9, 128), np.float32)
    for ct in range(4):
        Wb = Weff[ct * 128:(ct + 1) * 128]            # [128o, 128c]
        wdb = wd9o[ct * 128:(ct + 1) * 128]           # [128o, 9]
        for t in range(9):
            wfuse[:, ct, t, :] = (Wb * wdb[:, t:t + 1]).T
    sh["wfuse"] = _bf(wfuse)
    # bias terms: z = dwconv(qkv_nobias) + bd + bqkv * R(valid taps)
    bd512 = _f32(d["bd"])
    # tap index in TAPS order
    tix = {taps: i for i, taps in enumerate(TAPS)}
    dwbias = np.zeros((128, 4, 1), np.float32)
    cwed = np.zeros((2, 4, 128), np.float32)
    chrow = np.zeros((2, 4, 128), np.float32)
    corn = np.zeros((4, 4, 128), np.float32)
    for ct in range(4):
        wdb = wd9o[ct * 128:(ct + 1) * 128]
        bq_ = bqkv[ct * 128:(ct + 1) * 128]
        dwbias[:, ct, 0] = bd512[ct * 128:(ct + 1) * 128] + bq_ * wdb.sum(1)
        wsum = lambda tl: sum(wdb[:, tix[t]] for t in tl)
        cwed[0, ct, :] = -bq_ * wsum([(-1, -1), (0, -1), (1, -1)])
        cwed[1, ct, :] = -bq_ * wsum([(-1, 1), (0, 1), (1, 1)])
        chrow[0, ct, :] = -bq_ * wsum([(-1, -1), (-1, 0), (-1, 1)])
        chrow[1, ct, :] = -bq_ * wsum([(1, -1), (1, 0), (1, 1)])
        corn[0, ct, :] = bq_ * wdb[:, tix[(-1, -1)]]
        corn[1, ct, :] = bq_ * wdb[:, tix[(-1, 1)]]
        corn[2, ct, :] = bq_ * wdb[:, tix[(1, -1)]]
        corn[3, ct, :] = bq_ * wdb[:, tix[(1, 1)]]
    sh["dwbias"] = _f32(dwbias)
    sh["cwed"] = _bf(cwed)
    sh["chrow0"] = _bf(chrow[0:1])
    sh["chrow1"] = _bf(chrow[1:2])
    sh["corn0"] = _bf(corn[0:2])
    sh["corn1"] = _bf(corn[2:4])
    indw = np.zeros((2, 8), np.float32)
    indw[0, 0::2] = 1.0
    indw[1, 1::2] = 1.0
    sh["indw"] = _bf(indw)
    sh["ind2"] = _bf(np.eye(2))
    sh["onesr"] = _bf(np.ones((1, 128)))
    wp = _f32(d["wp"])[:, :, 0, 0] * _f32(d["gp"])[:, None]
    sh["wpT"] = _bf(wp.T.reshape(4, 128, 128).transpose(1, 0, 2))
    sh["bp6"] = _f32(d["bp"])[:, None] / 6.0
    wr = _f32(d["wr"])[:, :, 0, 0] * _f32(d["gr"])[:, None]
    sh["wrT"] = _bf(wr.T.reshape(2, 128, 256).transpose(1, 0, 2))
    sh["br"] = _f32(_f32(d["br"]).reshape(2, 128).T)
    wc2 = _f32(d["wc2"])[:, :, 0, 0] * _f32(d["gc2"])[:, None]
    sh["wc2T"] = _bf(wc2.T.reshape(2, 128, 256).transpose(1, 0, 2))
    sh["bc2"] = _f32(_f32(d["bc2"]).reshape(2, 128).T)
    wo = _f32(d["wo"])[:, :, 0, 0] * _f32(d["go"])[:, None]
    sh["woT"] = _bf(wo.T.reshape(2, 128, 128).transpose(1, 0, 2))
    sh["bo3"] = _f32(d["bo"])[:, None] + 3.0
    return sh


def _prep_core(x_b):
    xp = np.zeros((64, H + 2, WP), np.float32)
    xp[:, 1:129, 1:129] = x_b
    flat = xp.reshape(64, NPAD)
    shift = np.zeros_like(flat)
    shift[:, :-1] = flat[:, 1:]
    return _bf(np.concatenate([flat, shift], axis=0))


def kernel(**inputs):
    from concourse.bass_utils import run_bass_kernel_spmd
    sh = _prep_shared(inputs)
    nc = _NC_CACHE.get("nc")
    if nc is None:
        nc = build_nc()
        _NC_CACHE["nc"] = nc
    x = _f32(inputs["x"])
    in_maps = []
    for b in range(8):
        m = dict(sh)
        m["xstack"] = _prep_core(x[b])
        in_maps.append(m)
    res = run_bass_kernel_spmd(nc, in_maps, core_ids=list(range(8)),
                               trace=bool(int(os.environ.get("KTRACE", "0"))))
    if res.exec_time_ns is not None:
        print(f"HW exec time: {res.exec_time_ns} ns")
    _NC_CACHE["last"] = res
    out = np.stack([r["out"].reshape(128, 128, 128) for r in res.results])
    return out.astype(np.float32)


if __name__ == "__main__":
    build_nc()
    print("built ok")
